# revision 1
# baseline (speedup 1.0000x reference)
"""Sliding-window GQA attention on 8 trn2 NeuronCores.

Sharding: 8 cores = 2 batches x 4 KV groups. Core c=(4*b+g) handles batch b
and query heads [4g, 4g+4) (which share kv head g). Each core computes a
partial output x_b-attention-wo_g; the host sums the 4 group partials per
batch (the wo contraction is split across groups).

Dtypes: projections run fp16 (x, wq/wk/wv converted on host); the wo
contraction runs fp32r on fp32 O^T. The attention core (S, P, P@V) is fp16
(full PE rate at any N). V is projected directly into natural [token, d]
layout — the xT slab is the stationary operand — so there is no vT buffer
and no V transpose at all; the freed PSUM bank deepens the shared
accumulator pool (psX bufs=3).

Schedule: projection chunks (512 tokens) interleave with attention blocks —
after chunk n, query blocks 4n..4n+3 have all their keys, so 4 attention
steps run between chunks and the PE never starves on the x DMA or on the
softmax (Act/DVE) chains. The P^T XBAR for block i is issued in step i+1
and consumed by P@V in step i+2, keeping DMA latency off the critical path.

st=True (default) selects the S^T-oriented attention core: kT chunks are
the stationary operand (per-cell sT[k,q], no P transposes); lsum comes from
a ones-matvec sharing the eT stationary load with the P@V matmul; the
softmax normalize runs on 128-wide O rows (q on partitions) and only 4
O-transposes/block remain. HW-verified at rel 1.692e-2; models 201.4us vs
208.3us for the P-transpose path (st=False). NB: PSUM accumulation regions
sharing a bank must each complete before another region's start=True
(bank-granular pending-zero) — hence the h-outer/c-inner loop in
attn_back_st.
"""
import os
import sys

sys.path.insert(0, "/opt/trn_rl_repo")

import numpy as np

import concourse.bass as bass
import concourse.tile as tile
from concourse import bacc, mybir
from concourse.bass_utils import run_bass_kernel_spmd

B, L, DIM = 2, 2048, 2048
NH, NKV, HD = 16, 4, 128
W = 512
NHL = 4          # query heads per core
GDIM = NHL * HD  # 512 head-dims per core
SCALE = float(HD) ** -0.5
MASKVAL = -60000.0
KC = DIM // 128  # contraction chunks for projections
NB = L // 128    # query blocks
TOK = 512        # token chunk (N of projection matmuls)
NT = L // TOK    # 4 chunks
KH = KC // 2     # 8 contraction chunks per half slab
F32 = mybir.dt.float32
F32R = mybir.dt.float32r
F16 = mybir.dt.float16

_built = {}
last_results = None


def _key_range(i):
    """Keys needed by query block i: [klo, klo+wk)."""
    if i < 4:
        return 0, 128 * (i + 1)
    return 128 * (i - 4), 640


def _build(reps=1, simsafe=False, dmat="", orow="act", fuse=True, pdt="f16", st=True):
    key = (reps, simsafe, dmat, orow, fuse, pdt, st)
    PDT = F16 if pdt == "f16" else F32R
    if key in _built:
        return _built[key]

    nc = bacc.Bacc("TRN2", target_bir_lowering=False, debug=False,
                   enable_asserts=False)
    xT = nc.dram_tensor("xT", [DIM, L], PDT, kind="ExternalInput").ap()
    wq = nc.dram_tensor("wq", [DIM, GDIM], PDT, kind="ExternalInput").ap()
    wk = nc.dram_tensor("wk", [DIM, HD], PDT, kind="ExternalInput").ap()
    wv = nc.dram_tensor("wv", [DIM, HD], PDT, kind="ExternalInput").ap()
    wo = nc.dram_tensor("wo", [GDIM, DIM], F32R, kind="ExternalInput").ap()
    out = nc.dram_tensor("out", [L, DIM], F32, kind="ExternalOutput").ap()

    lag = 2 if "p" in dmat else 1
    psx_bufs = 4 if "p" in dmat else 3

    with tile.TileContext(nc) as tc:
      with tc.tile_pool(name="persist", bufs=1) as pers:
        # --- constants ---
        ident16 = pers.tile([128, 128], F16, tag="ident16")
        nc.gpsimd.memset(ident16, 0.0)
        nc.gpsimd.affine_select(
            out=ident16, in_=ident16, compare_op=mybir.AluOpType.not_equal,
            fill=1.0, base=0, pattern=[[-1, 128]], channel_multiplier=1)
        # additive triangle masks, fp16 (applied via PE matmul pre-write):
        # wide layout [up(128) | zeros(512)] / [zeros(512) | lo(128)] so the
        # simsafe variant can cover a whole accumulation extent
        maskw = pers.tile([128, 2, 128], F16, tag="maskw")
        nc.gpsimd.memset(maskw, 0.0)
        up, lo = maskw[:, 0, :], maskw[:, 1, :]
        mask = [up, lo]
        nc.gpsimd.affine_select(
            out=up, in_=up, compare_op=mybir.AluOpType.is_ge,
            fill=MASKVAL, base=0, pattern=[[1, 128]], channel_multiplier=-1)
        nc.gpsimd.affine_select(
            out=lo, in_=lo, compare_op=mybir.AluOpType.is_ge,
            fill=MASKVAL, base=0, pattern=[[-1, 128]], channel_multiplier=1)

        ones16 = pers.tile([128, 1], F16, tag="ones16")
        nc.gpsimd.memset(ones16, 1.0)

        # --- persistent tensors ---
        qT = [pers.tile([128, L], F16, tag=f"qT{m}", name=f"qT{m}")
              for m in range(NHL)]
        kT = pers.tile([128, L], F16, tag="kT")
        vnat = pers.tile([128, KC, 128], F16, tag="vnat")

        def mask_mm(dst, which, stop):
            """Add a triangle mask on top of already-written S values in
            PSUM (addition commutes, so mask-after-S == S-after-mask, and
            every PSUM region's first write carries start=True)."""
            nc.tensor.matmul(dst, ident16, mask[which],
                             start=False, stop=stop, skip_group_check=True)

        for _rep in range(reps):
            with tc.tile_pool(name="paw", bufs=1) as paw, \
                 tc.tile_pool(name="pax", bufs=2) as pax, \
                 tc.tile_pool(name="pb", bufs=2) as pb, \
                 tc.tile_pool(name="pco", bufs=2) as pco, \
                 tc.tile_pool(name="psX", bufs=psx_bufs, space="PSUM") as psX, \
                 tc.tile_pool(name="psT", bufs=1, space="PSUM") as psT, \
                 tc.tile_pool(name="psS", bufs=2, space="PSUM") as psS:
                wq_sb = paw.tile([128, KC, GDIM], PDT, tag="wq")
                wk_sb = paw.tile([128, KC, HD], PDT, tag="wk")
                wv_sb = paw.tile([128, KC, HD], PDT, tag="wv")
                wo_sb = paw.tile([128, NHL, DIM], F32R, tag="wo")
                wq_r = wq.rearrange("(kc p) n -> p kc n", p=128)
                wk_r = wk.rearrange("(kc p) n -> p kc n", p=128)
                wv_r = wv.rearrange("(kc p) n -> p kc n", p=128)
                xT_r = xT.rearrange("(kc p) t -> p kc t", p=128)

                def load_half(n, half, splits=2):
                    # split-slab DMAs into one half-slab tile so the first
                    # matmuls unblock after a fraction of the data (the very
                    # first slab uses finer splits to cut kernel warmup)
                    xh = pax.tile([128, KH, TOK], PDT, tag="x", name="xh")
                    step = KH // splits
                    for q in range(splits):
                        ks = np.s_[:, half * KH + step * q:
                                   half * KH + step * (q + 1),
                                   n * TOK:(n + 1) * TOK]
                        nc.sync.dma_start(
                            out=xh[:, step * q:step * (q + 1), :],
                            in_=xT_r[ks])
                    return xh

                slabs = {}

                def proj_chunk(n):
                    if n == 0:
                        # first wq quarter + both x halves first so the
                        # m-loop's first matmuls unblock asap; remaining
                        # weights and wo after
                        ksl0 = np.s_[:, 0:4, :]
                        nc.sync.dma_start(out=wq_sb[ksl0], in_=wq_r[ksl0])
                        slabs[(0, 0)] = load_half(0, 0, splits=4)
                        slabs[(0, 1)] = load_half(0, 1)
                        for q4 in range(1, 4):
                            ksl = np.s_[:, 4 * q4:4 * (q4 + 1), :]
                            nc.sync.dma_start(out=wq_sb[ksl], in_=wq_r[ksl])
                        for q4 in range(4):
                            ksl = np.s_[:, 4 * q4:4 * (q4 + 1), :]
                            nc.sync.dma_start(out=wk_sb[ksl], in_=wk_r[ksl])
                            nc.sync.dma_start(out=wv_sb[ksl], in_=wv_r[ksl])
                        nc.sync.dma_start(
                            out=wo_sb,
                            in_=wo.rearrange("(kc p) n -> p kc n", p=128))
                    x_lo = slabs.pop((n, 0))
                    x_hi = slabs.pop((n, 1))
                    for m in range(NHL + 1):
                        acc = psX.tile([128, TOK], F32, tag="acc", name="acc")
                        for kc in range(KC):
                            if m < NHL:
                                lhsT = wq_sb[:, kc, 128 * m:128 * (m + 1)]
                            else:
                                lhsT = wk_sb[:, kc, :]
                            xh = x_lo if kc < KH else x_hi
                            nc.tensor.matmul(acc, lhsT, xh[:, kc % KH, :],
                                             start=(kc == 0),
                                             stop=(kc == KC - 1))
                        sl = np.s_[:, n * TOK:(n + 1) * TOK]
                        if m < NHL:
                            nc.vector.tensor_copy(qT[m][sl], acc)
                        else:
                            nc.vector.tensor_copy(kT[sl], acc)
                        # prefetch next chunk's slabs midway through
                        if m == 0 and n + 1 < NT:
                            slabs[(n + 1, 0)] = load_half(n + 1, 0)
                        if m == 2 and n + 1 < NT:
                            slabs[(n + 1, 1)] = load_half(n + 1, 1)
                    # V directly in natural [token, d] layout: the xT slab is
                    # the stationary operand, wv the moving one — no vT, no
                    # transposes. 4 token-block regions per psum slot,
                    # region-outer (bank-granular pending-zero).
                    vacc = psX.tile([128, TOK], F32, tag="acc", name="vacc")
                    for tb in range(4):
                        for kc in range(KC):
                            xh = x_lo if kc < KH else x_hi
                            nc.tensor.matmul(
                                vacc[:, 128 * tb:128 * (tb + 1)],
                                xh[:, kc % KH, 128 * tb:128 * (tb + 1)],
                                wv_sb[:, kc, :],
                                start=(kc == 0), stop=(kc == KC - 1),
                                skip_group_check=True)
                    nc.vector.tensor_copy(vnat[:, 4 * n:4 * (n + 1), :], vacc)

                ptq_saved = {}
                p_saved = {}
                oT_saved = {}

                def emit_ptrans(i):
                    # XBAR transposes for block i's P tile; deferred one
                    # attention step so Act.SEQ never waits on the DVE
                    # normalize chain, and consumed another step later so
                    # the XBAR latency stays off the critical path
                    klo, wkk = _key_range(i)
                    nch = wkk // 128
                    p_all = p_saved.pop(i)
                    ptq = pb.tile([128, NHL, 5, 128], F16, tag="ptq",
                                  name="ptq")
                    if nch == 5:
                        # all heads + chunks in one XBAR instruction:
                        # in free index h*640 + c*128 + k -> out[:, h, c, :]
                        nc.scalar.dma_start(out=ptq, in_=p_all,
                                            transpose=True)
                    else:
                        for h in range(NHL):
                            nc.scalar.dma_start(out=ptq[:, h, 0:nch, :],
                                                in_=p_all[:, h, :wkk],
                                                transpose=True)
                    ptq_saved[i] = ptq

                eT_saved = {}
                ls_saved = {}

                def attn_front_st(i):
                    # S^T orientation: sT[k, c, q] per head; exp -> eT fp16;
                    # no P transposes (O is transposed instead, 4/block)
                    klo, wkk = _key_range(i)
                    nch = wkk // 128
                    eTs = []
                    for h in range(NHL):
                        st_ps = psS.tile([128, 5, 128], F32, tag="s",
                                         name="st_ps")
                        for c in range(nch):
                            kg = klo + 128 * c
                            diag = kg == 128 * i
                            edge = i >= 4 and c == 0
                            nc.tensor.matmul(
                                st_ps[:, c, :], kT[:, kg:kg + 128],
                                qT[h][:, 128 * i:128 * (i + 1)],
                                start=True, stop=not (diag or edge),
                                skip_group_check=True)
                            if diag:
                                # diagonal cell: invalid where k > q
                                mask_mm(st_ps[:, c, :], 0, stop=True)
                            elif edge:
                                # window edge: invalid where q_l > k_l
                                mask_mm(st_ps[:, c, :], 1, stop=True)
                        eT = pb.tile([128, 5, 128], F16, tag=f"eT{h}",
                                     name="eT", bufs=2)
                        nc.scalar.activation(
                            out=eT[:, 0:nch, :], in_=st_ps[:, 0:nch, :],
                            func=mybir.ActivationFunctionType.Exp,
                            scale=SCALE)
                        eTs.append(eT)
                    eT_saved[i] = eTs

                def attn_back_st(i):
                    klo, wkk = _key_range(i)
                    nch = wkk // 128
                    eTs = eT_saved.pop(i)
                    o_ps = psX.tile([128, 512], F32, tag="acc", name="o_ps")
                    ls_ps = psX.tile([128, 512], F32, tag="acc",
                                     name="ls_ps")
                    for h in range(NHL):
                        # h outer: each PSUM region's accumulation completes
                        # before another region's start=True marks the bank
                        # pending-zero. eT cell is the stationary operand for
                        # both the PV matmul and the lsum matvec (one load).
                        for c in range(nch):
                            nc.tensor.matmul(
                                o_ps[:, 128 * h:128 * (h + 1)],
                                eTs[h][:, c, :],
                                vnat[:, klo // 128 + c, :],
                                start=(c == 0), stop=(c == nch - 1),
                                skip_group_check=True)
                            nc.tensor.matmul(
                                ls_ps[:, h:h + 1], eTs[h][:, c, :], ones16,
                                start=(c == 0), stop=(c == nch - 1),
                                skip_group_check=True)
                    linv = pb.tile([128, 4], F32, tag="linv2", name="linv",
                                   bufs=2)
                    nc.vector.reciprocal(linv, ls_ps[:, 0:4])
                    o_n = pb.tile([128, NHL, 128], F16, tag="on", name="o_n",
                                  bufs=2)
                    for h in range(NHL):
                        nc.vector.tensor_scalar_mul(
                            o_n[:, h, :], o_ps[:, 128 * h:128 * (h + 1)],
                            linv[:, h:h + 1])
                    # O^T via 4 PE transposes into one bank, one copy out
                    ot_ps = psT.tile([128, 512], F16, tag="t", name="ot_ps")
                    for h in range(NHL):
                        nc.tensor.transpose(ot_ps[:, 128 * h:128 * (h + 1)],
                                            o_n[:, h, :], ident16)
                    oT = pb.tile([128, NHL, 128], F32R, tag="oT", name="oT",
                                 bufs=4)
                    nc.vector.tensor_copy(oT, ot_ps)
                    oT_saved[i] = oT

                def attn_front(i):
                    klo, wkk = _key_range(i)
                    nch = wkk // 128
                    if i >= 1 and "p" in dmat:
                        emit_ptrans(i - 1)
                    p_all = pb.tile([128, NHL, 640], F16, tag="p",
                                    name="p_all")
                    for h in range(NHL):
                        s_ps = psS.tile([128, 640], F32, tag="s", name="s_ps")
                        if i < 4:
                            # S with start=True, then the causal triangle
                            # added on the last 128 cols
                            nc.tensor.matmul(
                                s_ps[:, 0:wkk],
                                qT[h][:, 128 * i:128 * (i + 1)],
                                kT[:, klo:klo + wkk],
                                start=True, stop=False, skip_group_check=True)
                            mask_mm(s_ps[:, wkk - 128:wkk], 1, stop=True)
                        else:
                            nc.tensor.matmul(
                                s_ps[:, 0:512],
                                qT[h][:, 128 * i:128 * (i + 1)],
                                kT[:, klo:klo + 512],
                                start=True, stop=False, skip_group_check=True)
                            mask_mm(s_ps[:, 0:128], 0, stop=False)
                            nc.tensor.matmul(
                                s_ps[:, 512:640],
                                qT[h][:, 128 * i:128 * (i + 1)],
                                kT[:, klo + 512:klo + 640],
                                start=True, stop=False, skip_group_check=True)
                            mask_mm(s_ps[:, 512:640], 1, stop=True)
                        e_sb = pb.tile([128, 640], F32, tag="e",
                                       name="e_sb", bufs=4)
                        lsum = pb.tile([128, 1], F32, tag=f"l{h}", name="lsum",
                                       bufs=2)
                        nc.scalar.activation(
                            out=e_sb[:, :wkk], in_=s_ps[:, :wkk],
                            func=mybir.ActivationFunctionType.Exp,
                            scale=SCALE, accum_out=lsum)
                        linv = pb.tile([128, 1], F32, tag=f"li{h}",
                                       name="linv", bufs=2)
                        nc.vector.reciprocal(linv, lsum)
                        # NB: DVE, not gpsimd — gpsimd tensor_scalar costs
                        # ~7us/op on HW and serializes the block pipeline
                        nc.vector.tensor_scalar_mul(p_all[:, h, :wkk],
                                                    e_sb[:, :wkk], linv)
                    p_saved[i] = p_all
                    if "p" not in dmat:
                        ptq = pb.tile([128, NHL, 5, 128], F16, tag="ptq",
                                      name="ptq")
                        for c in range(nch):
                            t_ps = psT.tile([128, 512], F16, tag="t",
                                            name="t_ps")
                            for h in range(NHL):
                                nc.tensor.transpose(
                                    t_ps[:, 128 * h:128 * (h + 1)],
                                    p_all[:, h, 128 * c:128 * (c + 1)],
                                    ident16)
                            nc.vector.tensor_copy(ptq[:, :, c, :], t_ps)
                        p_saved.pop(i, None)
                        ptq_saved[i] = ptq

                def attn_back(i):
                    klo, wkk = _key_range(i)
                    nch = wkk // 128
                    ptq = ptq_saved.pop(i)
                    # O^T accumulation over key chunks (all heads at once)
                    o_ps = psX.tile([128, 512], F32, tag="acc", name="o_ps")
                    for c in range(nch):
                        nc.tensor.matmul(
                            o_ps, vnat[:, klo // 128 + c, :], ptq[:, :, c, :],
                            start=(c == 0), stop=(c == nch - 1))
                    # rolling O^T buffer: consumed by out_tile 3 steps later
                    oT = pb.tile([128, NHL, 128], F32R, tag="oT", name="oT",
                                 bufs=4)
                    nc.vector.tensor_copy(oT, o_ps)
                    oT_saved[i] = oT

                def out_tile(tt):
                    oT = oT_saved.pop(tt)
                    for half in range(2):
                        o_row = pco.tile([128, DIM // 2], F32, tag="orow",
                                         name="o_row")
                        for sub in range(2):
                            nn = 2 * half + sub
                            acc = psX.tile([128, 512], F32, tag="acc",
                                           name="acc")
                            for kc in range(NHL):
                                nc.tensor.matmul(
                                    acc, oT[:, kc, :],
                                    wo_sb[:, kc, 512 * nn:512 * (nn + 1)],
                                    start=(kc == 0), stop=(kc == NHL - 1))
                            on_act = (orow == "act" or
                                      (orow == "mix" and nn % 2 == 1))
                            if on_act:
                                nc.scalar.activation(
                                    out=o_row[:, 512 * sub:512 * (sub + 1)],
                                    in_=acc,
                                    func=mybir.ActivationFunctionType.Copy)
                            else:
                                nc.vector.tensor_copy(
                                    o_row[:, 512 * sub:512 * (sub + 1)], acc)
                        nc.sync.dma_start(
                            out=out[128 * tt:128 * (tt + 1),
                                    1024 * half:1024 * (half + 1)],
                            in_=o_row)

                def att_step(i):
                    (attn_front_st if st else attn_front)(i)
                    if i >= lag:
                        (attn_back_st if st else attn_back)(i - lag)
                    if i >= lag + 1:
                        out_tile(i - lag - 1)

                if fuse:
                    for n in range(NT):
                        proj_chunk(n)
                        for i in range(4 * n, 4 * (n + 1)):
                            att_step(i)
                else:
                    for n in range(NT):
                        proj_chunk(n)
                    for i in range(NB):
                        att_step(i)
                # drain
                if "p" in dmat and not st:
                    emit_ptrans(NB - 1)
                for i in range(NB - lag, NB):
                    (attn_back_st if st else attn_back)(i)
                for tt in range(NB - lag - 1, NB):
                    out_tile(tt)

    nc.compile()
    _built[key] = nc
    return nc


def prep_inputs(x, wq, wk, wv, wo, pdt="f16"):
    """Full fp32 inputs -> per-core input maps (8 cores)."""
    npdt = np.float16 if pdt == "f16" else np.float32
    x = np.asarray(x, dtype=np.float32)
    xT = [np.ascontiguousarray(x[b].T.astype(npdt)) for b in range(B)]
    wqh = np.asarray(wq, dtype=npdt)
    wkh = np.asarray(wk, dtype=npdt)
    wvh = np.asarray(wv, dtype=npdt)
    woh = np.asarray(wo, dtype=np.float32)
    in_maps = []
    for c in range(8):
        b, g = c // 4, c % 4
        in_maps.append({
            "xT": xT[b],
            "wq": np.ascontiguousarray(wqh[:, GDIM * g:GDIM * (g + 1)]),
            "wk": np.ascontiguousarray(wkh[:, HD * g:HD * (g + 1)]),
            "wv": np.ascontiguousarray(wvh[:, HD * g:HD * (g + 1)]),
            "wo": np.ascontiguousarray(woh[GDIM * g:GDIM * (g + 1), :]),
        })
    return in_maps


def kernel(x, wq, wk, wv, wo):
    global last_results
    nc = _build()
    in_maps = prep_inputs(x, wq, wk, wv, wo)
    res = run_bass_kernel_spmd(nc, in_maps, list(range(8)))
    last_results = res
    out = np.empty((B, L, DIM), dtype=np.float32)
    for b in range(B):
        acc = np.zeros((L, DIM), dtype=np.float64)
        for g in range(4):
            acc += res.results[4 * b + g]["out"]
        out[b] = acc.astype(np.float32)
    return out

